# revision 4
# baseline (speedup 1.0000x reference)
"""Bass/Trainium2 kernel for nn_KernelizedAttentionResBlock.

Sharding: n-token sharded attention (each of 8 cores owns 128 rows of n for
all batches), one AllGather of x^T (n-major, fp16 payload), m-sharded FFN,
host sums the 8 partial FFN outputs and adds the residual/bias.

Levers:
- K/V + weights fp16: the kernel is DMA-bound at ~360 GB/s/core, so halving
  HBM traffic halves phase-2 time.
- Gaussian kernel in ONE activation pass: Derivative_Erf(rs*K - mu*rs) =
  2/sqrt(pi) * exp(-0.5 (K-mu)^2/(sigma^2+1e-8)); sqrt(pi)/2 folded into V
  on the host. (tensor_tensor_reduce would fuse the reduce too but wedges
  the device, so S*V and the D-reduce are separate ops.)
- The 32 multiplies/reduces are spread over DVE, Pool (gpsimd multiplies)
  and the Act engine (Identity with accum_out, which stays inside the
  erf_derivative act table), keeping every engine under the DMA pace.
- Only {Square, Tanh, Derivative_Erf, Sigmoid, Identity, Copy} activations:
  3 act-table loads, all off the critical path (a dummy Sigmoid pinned
  behind phase 2 preloads the FFN table during the collective). rsqrt is
  computed on the DVE via the 0x5f3759df bit trick + 1 Newton step (the
  Ln/Exp route would thrash activation tables).
- Both LayerNorms are folded into the adjacent matmuls in transposed
  layout: ln(v)^T @ W = rstd_b * (v^T @ W - mean_b * rowsum(W)); the
  rank-1 mean correction is an extra matmul accumulated into the same
  PSUM and rstd_b is a PE-broadcast row multiplied in afterwards (via an
  SBUF copy: tensor ops may read only one PSUM operand).
- Head tensors first on the SP queue so phase 0/1 starts immediately; FFN
  weights issued after cc_in so their transfer hides inside the collective.
"""
import os
import sys

sys.path.insert(0, "/opt/trn_rl_repo")

import numpy as np

N = 1024
B = 32
D = 1024
M = 4096
NCORES = 8
NSL = N // NCORES
MSL = M // NCORES
MCH = MSL // 128
NB = 4
LN_EPS = 1e-5
MAGIC1 = 0x5F3759E0  # 0x5f3759df + 1 (for MAGIC - x == ~x + MAGIC+1)
SQH = float(np.sqrt(0.5))

_built = {}
last_results = None


def _build_module():
    if "nc" in _built:
        return _built["nc"]

    import concourse.bacc as bacc
    import concourse.mybir as mybir
    import concourse.tile as tile

    AF = mybir.ActivationFunctionType
    ALU = mybir.AluOpType
    f32 = mybir.dt.float32
    f16 = mybir.dt.float16
    i32 = mybir.dt.int32

    nc = bacc.Bacc(trn_type="TRN2", num_devices=NCORES)

    Kd = nc.dram_tensor("Ks", (NSL, B, D), f16, kind="ExternalInput")
    Vd = nc.dram_tensor("Vs", (NSL, B, D), f16, kind="ExternalInput")
    # HEAD packs [-mu_b | sig_b | Q^T slice] along the free dim
    HEAD = nc.dram_tensor("HEAD", (128, 2 + B), f32, kind="ExternalInput")
    QTF = nc.dram_tensor("QTF", (128, NCORES, B), f16, kind="ExternalInput")
    MSW = nc.dram_tensor("MSW", (128, 2, NCORES, NSL), f16,
                         kind="ExternalInput")
    MS2 = nc.dram_tensor("MS2", (1, 2 * NSL), f32, kind="ExternalInput")
    W1T = nc.dram_tensor("W1T", (128, NCORES, MSL), f16, kind="ExternalInput")
    FFNB = nc.dram_tensor("FFNB", (128, MCH), f32, kind="ExternalInput")
    W1S = nc.dram_tensor("W1S", (1, MSL), f32, kind="ExternalInput")
    W2T = nc.dram_tensor("W2T", (128, MCH, N), f16, kind="ExternalInput")

    XTd = nc.dram_tensor("XT", (NSL, B), f32, kind="ExternalOutput")
    HPd = nc.dram_tensor("HP", (N, B), f32, kind="ExternalOutput")

    cc_in = nc.dram_tensor("cc_in", (NSL, B), f16, kind="Internal")
    cc_out = nc.dram_tensor(
        "cc_out", (N, B), f16, kind="Internal", addr_space="Shared"
    )

    def rsqrt_newton(pool, tag, v, steps, init, final_scale=1.0):
        """rsqrt(v) on the DVE: `init` is a same-shape f32 starting guess
        builder; 2 Newton steps; final_scale folded into the last step."""
        shape = list(v.shape)
        y = init
        for s in range(steps):
            k = final_scale if s == steps - 1 else 1.0
            t1 = pool.tile(shape, f32, tag=f"{tag}t1")
            nc.vector.tensor_mul(t1[:], y, y)
            t2 = pool.tile(shape, f32, tag=f"{tag}t2")
            nc.vector.tensor_mul(t2[:], t1[:], v[:])
            t3 = pool.tile(shape, f32, tag=f"{tag}t3")
            nc.vector.tensor_scalar(t3[:], t2[:], -0.5 * k, 1.5 * k,
                                    op0=ALU.mult, op1=ALU.add)
            yn = pool.tile(shape, f32, tag=f"{tag}n{s}")
            nc.vector.tensor_mul(yn[:], y, t3[:])
            y = yn[:]
        return y

    def rsqrt_bit(pool, tag, v, final_scale=1.0):
        """Full-range rsqrt: quake init + 2 Newton steps."""
        shape = list(v.shape)
        sh = pool.tile(shape, i32, tag=f"{tag}sh")
        nc.vector.tensor_scalar(sh[:], v[:].bitcast(i32), 1, None,
                                op0=ALU.logical_shift_right)
        y0i = pool.tile(shape, i32, tag=f"{tag}y0")
        nc.vector.tensor_scalar(y0i[:], sh[:], -1, MAGIC1 - 1,
                                op0=ALU.mult, op1=ALU.add)
        return rsqrt_newton(pool, tag, v, 1, y0i[:].bitcast(f32),
                            final_scale)

    with tile.TileContext(nc) as tc:
        with tc.tile_pool(name="const", bufs=1) as cst, \
             tc.tile_pool(name="small", bufs=1) as sm, \
             tc.tile_pool(name="kv", bufs=6) as kv, \
             tc.tile_pool(name="scr", bufs=4) as scr, \
             tc.tile_pool(name="psum", bufs=1, space="PSUM") as ps:

            # ---- head tensors first, then the K/V stream (SP queue) ---
            head = cst.tile([128, 2 + B], f32)
            nc.sync.dma_start(head[:], HEAD[:])
            qtf = cst.tile([128, NCORES, B], f16)
            nc.sync.dma_start(qtf[:], QTF[:])
            msw = cst.tile([128, 2, NCORES, NSL], f16)
            nc.sync.dma_start(msw[:], MSW[:])
            ms2 = cst.tile([1, 2 * NSL], f32)
            nc.sync.dma_start(ms2[:], MS2[:])

            BLOCKS = [(i * NB, NB) for i in range(B // NB - 1)]
            BLOCKS += [(B - NB, NB // 2), (B - NB // 2, NB // 2)]
            kts, vts = [], []
            for b0, nb in BLOCKS:
                kt = kv.tile([NSL, nb, D], f16, tag=f"kt{nb}")
                nc.sync.dma_start(kt[:], Kd[:, b0:b0 + nb, :])
                vt = kv.tile([NSL, nb, D], f16, tag=f"vt{nb}")
                nc.sync.dma_start(vt[:], Vd[:, b0:b0 + nb, :])
                kts.append(kt)
                vts.append(vt)

            # ---- head tensors on the Pool queue -----------------------
            nmb = head[:, 0:1]
            sb1 = head[:, 1:2]
            qts = head[:, 2:2 + B]

            ones16 = cst.tile([128, 1], f16)
            nc.vector.memset(ones16[:], 1.0)
            ones32 = cst.tile([128, 1], f32)
            nc.vector.memset(ones32[:], 1.0)
            ones_row = cst.tile([1, 128], f32)
            nc.vector.memset(ones_row[:], 1.0)

            # ---------- Phase 0/1: LN(Q) folded into mu/sigma -----------
            qsq = sm.tile([128, NCORES, B], f32)
            nc.scalar.activation(qsq[:], qtf[:], AF.Square)
            qs_ps = ps.tile([1, B], f32, tag="pA")
            for c in range(NCORES):
                nc.tensor.matmul(qs_ps[:], ones16[:], qtf[:, c, :],
                                 start=(c == 0), stop=(c == NCORES - 1))
            qs2_ps = ps.tile([1, B], f32, tag="pB")
            for c in range(NCORES):
                nc.tensor.matmul(qs2_ps[:], ones32[:], qsq[:, c, :],
                                 start=(c == 0), stop=(c == NCORES - 1))
            negmean = sm.tile([1, B], f32)
            nc.vector.tensor_scalar_mul(negmean[:], qs_ps[:], -1.0 / N)
            msq = sm.tile([1, B], f32)
            nc.vector.tensor_mul(msq[:], negmean[:], negmean[:])
            varq = sm.tile([1, B], f32)
            nc.vector.tensor_scalar(varq[:], qs2_ps[:], 1.0 / N, LN_EPS,
                                    op0=ALU.mult, op1=ALU.add)
            varq2 = sm.tile([1, B], f32)
            nc.vector.tensor_sub(varq2[:], varq[:], msq[:])
            # q-row variance is ~1 (Q ~ N(0,1)): linear init + 2 Newton
            y0q = sm.tile([1, B], f32)
            nc.vector.tensor_scalar(y0q[:], varq2[:], -0.5, 1.5,
                                    op0=ALU.mult, op1=ALU.add)
            rstdq = rsqrt_newton(sm, "rq", varq2, 1, y0q[:])
            RSTD0 = ps.tile([128, B], f32, tag="pC")
            nc.tensor.matmul(RSTD0[:], ones_row[:], rstdq,
                             start=True, stop=True)
            rstd0_sb = sm.tile([128, B], f32)
            nc.vector.tensor_scalar_mul(rstd0_sb[:], RSTD0[:], 1.0)

            mu_ps = ps.tile([NSL, B], f32, tag="pA")
            nc.tensor.matmul(mu_ps[:], ms2[:, 0:NSL], negmean[:],
                             start=True, stop=False)
            for c in range(NCORES):
                nc.tensor.matmul(mu_ps[:], msw[:, 0, c, :], qtf[:, c, :],
                                 start=False, stop=(c == NCORES - 1))
            zmu = sm.tile([NSL, B], f32)
            nc.vector.tensor_mul(zmu[:], mu_ps[:], rstd0_sb[:])
            negmu = sm.tile([NSL, B], f32)
            nc.scalar.activation(negmu[:], zmu[:], AF.Tanh,
                                 bias=nmb, scale=-1.0)

            sig_ps = ps.tile([NSL, B], f32, tag="pB")
            nc.tensor.matmul(sig_ps[:], ms2[:, NSL:2 * NSL], negmean[:],
                             start=True, stop=False)
            for c in range(NCORES):
                nc.tensor.matmul(sig_ps[:], msw[:, 1, c, :], qtf[:, c, :],
                                 start=False, stop=(c == NCORES - 1))
            zsig = sm.tile([NSL, B], f32)
            nc.vector.tensor_mul(zsig[:], sig_ps[:], rstd0_sb[:])
            s2 = sm.tile([NSL, B], f32)
            nc.scalar.activation(s2[:], zsig[:], AF.Square, bias=sb1)
            s2e = sm.tile([NSL, B], f32)
            nc.vector.tensor_scalar_add(s2e[:], s2[:], 1e-8)
            # rs = sqrt(0.5/(sigma^2+1e-8)) — full-range bit-trick rsqrt
            rs = rsqrt_bit(sm, "rs", s2e, final_scale=SQH)
            nmr = sm.tile([NSL, B], f32)
            nc.vector.tensor_mul(nmr[:], negmu[:], rs)

            # ---------- Phase 2: x^T = sum_D S*V' + Q^T -----------------
            # tensor_tensor_reduce wedges the device; use mult + reduce,
            # spreading 12 of 32 reduces onto the idle Pool engine.
            AX = mybir.AxisListType
            xT = sm.tile([NSL, B], f32)
            for blk, (b0, nb) in enumerate(BLOCKS):
                kt, vt = kts[blk], vts[blk]
                for bi in range(nb):
                    b = b0 + bi
                    es = scr.tile([NSL, D], f16, tag="es")
                    nc.scalar.activation(es[:], kt[:, bi, :],
                                         AF.Derivative_Erf,
                                         bias=nmr[:, b:b + 1],
                                         scale=rs[:, b:b + 1])
                    sv = scr.tile([NSL, D], f16, tag="sv")
                    meng = nc.gpsimd if b % 4 == 1 else nc.vector
                    meng.tensor_mul(sv[:], es[:], vt[:, bi, :])
                    if b % 4 == 3:
                        # Act engine reduce: Identity with accumulator out
                        # (identity is in the erf_derivative table: no load)
                        ad = scr.tile([NSL, D], f16, tag="ad")
                        nc.scalar.activation(ad[:], sv[:], AF.Identity,
                                             accum_out=xT[:, b:b + 1])
                    else:
                        nc.vector.reduce_sum(xT[:, b:b + 1], sv[:],
                                             axis=AX.X)
            xT2 = sm.tile([NSL, B], f32)
            nc.vector.tensor_add(xT2[:], xT[:], qts)

            xh16 = sm.tile([NSL, B], f16)
            nc.vector.tensor_scalar_mul(xh16[:], xT2[:], 1.0)
            nc.sync.dma_start(cc_in[:], xh16[:])
            nc.sync.dma_start(XTd[:], xT2[:])

            # preload the sigmoid act table during the collective window
            # (input xT2 pins it after phase 2 so the load can't be hoisted)
            sgd = sm.tile([NSL, 1], f32)
            nc.scalar.activation(sgd[:], xT2[:, 0:1], AF.Sigmoid)

            # FFN weights: transfers land inside the collective window.
            w1T = cst.tile([128, NCORES, MSL], f16)
            nc.sync.dma_start(w1T[:], W1T[:])
            ffnb = cst.tile([128, MCH], f32)
            nc.sync.dma_start(ffnb[:], FFNB[:])
            w1s = cst.tile([1, MSL], f32)
            nc.sync.dma_start(w1s[:], W1S[:])
            w2T = cst.tile([128, MCH, N], f16)
            nc.sync.dma_start(w2T[:], W2T[:])

            # ---------- Phase 3: AllGather x^T (n-major), LN, FFN -------
            nc.gpsimd.collective_compute(
                "AllGather", ALU.bypass,
                replica_groups=[list(range(NCORES))],
                ins=[cc_in[:]], outs=[cc_out[:]],
            )
            xg16 = sm.tile([128, NCORES, B], f16)
            nc.sync.dma_start(
                xg16[:], cc_out[:].rearrange("(c p) b -> p c b", p=128))
            xsq = sm.tile([128, NCORES, B], f32)
            nc.scalar.activation(xsq[:], xg16[:], AF.Square)
            s_ps = ps.tile([1, B], f32, tag="pA")
            for c in range(NCORES):
                nc.tensor.matmul(s_ps[:], ones16[:], xg16[:, c, :],
                                 start=(c == 0), stop=(c == NCORES - 1))
            s2_ps = ps.tile([1, B], f32, tag="pB")
            for c in range(NCORES):
                nc.tensor.matmul(s2_ps[:], ones32[:], xsq[:, c, :],
                                 start=(c == 0), stop=(c == NCORES - 1))
            negmx = sm.tile([1, B], f32)
            nc.vector.tensor_scalar_mul(negmx[:], s_ps[:], -1.0 / N)
            msqx = sm.tile([1, B], f32)
            nc.vector.tensor_mul(msqx[:], negmx[:], negmx[:])
            varx = sm.tile([1, B], f32)
            nc.vector.tensor_scalar(varx[:], s2_ps[:], 1.0 / N, LN_EPS,
                                    op0=ALU.mult, op1=ALU.add)
            varx2 = sm.tile([1, B], f32)
            nc.vector.tensor_sub(varx2[:], varx[:], msqx[:])
            rstdx = rsqrt_bit(sm, "rx", varx2)
            RSTD1 = ps.tile([128, B], f32, tag="pC")
            nc.tensor.matmul(RSTD1[:], ones_row[:], rstdx,
                             start=True, stop=True)
            rstd1_sb = sm.tile([128, B], f32)
            nc.vector.tensor_scalar_mul(rstd1_sb[:], RSTD1[:], 1.0)

            # FFN: h1 = (x@w1 - mean*w1sum)*rstd + b1 ; silu = z*sigmoid(z)
            g1_sb = sm.tile([128, MCH, B], f16)
            for mi in range(MCH):
                h1_ps = ps.tile([128, B], f32, tag=f"p{chr(68 + mi)}")
                nc.tensor.matmul(h1_ps[:], w1s[:, mi * 128:(mi + 1) * 128],
                                 negmx[:], start=True, stop=False)
                for c in range(NCORES):
                    nc.tensor.matmul(h1_ps[:],
                                     w1T[:, c, mi * 128:(mi + 1) * 128],
                                     xg16[:, c, :],
                                     start=False, stop=(c == NCORES - 1))
                zpre = sm.tile([128, B], f32, tag=f"zp_{mi}")
                nc.vector.tensor_mul(zpre[:], h1_ps[:], rstd1_sb[:])
                sg = sm.tile([128, B], f32, tag=f"sg_{mi}")
                nc.scalar.activation(sg[:], zpre[:], AF.Sigmoid,
                                     bias=ffnb[:, mi:mi + 1])
                z = sm.tile([128, B], f32, tag=f"z_{mi}")
                nc.vector.tensor_scalar_add(z[:], zpre[:],
                                            ffnb[:, mi:mi + 1])
                nc.vector.tensor_mul(g1_sb[:, mi, :], z[:], sg[:])

            hp_sb = sm.tile([128, NCORES, B], f32)
            hpv = HPd[:].rearrange("(jn p) b -> p jn b", p=128)
            for jn in range(NCORES):
                hp_ps = ps.tile([128, B], f32, tag=f"p{chr(68 + jn % 4)}")
                for mi in range(MCH):
                    nc.tensor.matmul(hp_ps[:],
                                     w2T[:, mi, jn * 128:(jn + 1) * 128],
                                     g1_sb[:, mi, :],
                                     start=(mi == 0), stop=(mi == MCH - 1))
                if jn % 2 == 0:
                    nc.scalar.copy(hp_sb[:, jn, :], hp_ps[:])
                else:
                    nc.vector.tensor_scalar_mul(hp_sb[:, jn, :], hp_ps[:],
                                                1.0)
                if jn == 3:
                    nc.sync.dma_start(hpv[:, 0:4, :], hp_sb[:, 0:4, :])
            nc.sync.dma_start(hpv[:, 4:8, :], hp_sb[:, 4:8, :])

    nc.finalize()
    _built["nc"] = nc
    return nc


def kernel(**inputs):
    from concourse.bass_utils import run_bass_kernel_spmd

    global last_results

    Q = np.asarray(inputs["Q"], dtype=np.float32)
    K = np.asarray(inputs["K"], dtype=np.float32)
    V = np.asarray(inputs["V"], dtype=np.float32)
    mu_w = np.asarray(inputs["mu_w"], dtype=np.float32)
    mu_b = np.asarray(inputs["mu_b"], dtype=np.float32)
    sigma_w = np.asarray(inputs["sigma_w"], dtype=np.float32)
    sigma_b = np.asarray(inputs["sigma_b"], dtype=np.float32)
    ffn_w1 = np.asarray(inputs["ffn_w1"], dtype=np.float32)
    ffn_b1 = np.asarray(inputs["ffn_b1"], dtype=np.float32)
    ffn_w2 = np.asarray(inputs["ffn_w2"], dtype=np.float32)
    ffn_b2 = np.asarray(inputs["ffn_b2"], dtype=np.float32)
    ln_ff_g = np.asarray(inputs["ln_ff_g"], dtype=np.float32)
    ln_ff_b = np.asarray(inputs["ln_ff_b"], dtype=np.float32)
    ln_q_g = np.asarray(inputs["ln_q_g"], dtype=np.float32)
    ln_q_b = np.asarray(inputs["ln_q_b"], dtype=np.float32)

    # ---- Host-side exact folds of LN affine params into next matmuls ----
    mu_wf = mu_w * ln_q_g[None, :]
    mu_bf = mu_b + mu_w @ ln_q_b
    sig_wf = sigma_w * ln_q_g[None, :]
    sig_bf = sigma_b + sigma_w @ ln_q_b
    w1f = ffn_w1 * ln_ff_g[None, :]
    b1f = ffn_b1 + ffn_w1 @ ln_ff_b
    w1sum = w1f.sum(axis=1)
    musum = mu_wf.sum(axis=1)
    sigsum = sig_wf.sum(axis=1)

    # Device computes S*V' with S = Derivative_Erf(u) = 2/sqrt(pi)*exp(-u^2)
    Vs = (V * (np.sqrt(np.pi) / 2.0)).astype(np.float16)
    Kh = K.astype(np.float16)

    QT = np.ascontiguousarray(Q.T)                    # (N, B)
    qtf = QT.reshape(NCORES, 128, B).transpose(1, 0, 2)
    muwT = np.ascontiguousarray(mu_wf.T)              # (N, N)  [jn, j]
    sigwT = np.ascontiguousarray(sig_wf.T)
    w1T = np.ascontiguousarray(w1f.T)                 # (N, M)
    w2T = np.ascontiguousarray(ffn_w2.T)              # (M, N)

    nc = _build_module()

    in_maps = []
    for c in range(NCORES):
        jsl = slice(c * NSL, (c + 1) * NSL)
        msl = slice(c * MSL, (c + 1) * MSL)
        head = np.concatenate([
            (-mu_bf[jsl]).reshape(NSL, 1),
            sig_bf[jsl].reshape(NSL, 1),
            QT[jsl, :],
        ], axis=1)
        msw = np.stack([
            muwT[:, jsl].reshape(NCORES, 128, NSL).transpose(1, 0, 2),
            sigwT[:, jsl].reshape(NCORES, 128, NSL).transpose(1, 0, 2),
        ], axis=1)                                    # (128, 2, 8, NSL)
        ms2 = np.concatenate([musum[jsl], sigsum[jsl]]).reshape(1, 2 * NSL)
        in_maps.append({
            "Ks": np.ascontiguousarray(Kh[:, jsl, :].transpose(1, 0, 2)),
            "Vs": np.ascontiguousarray(Vs[:, jsl, :].transpose(1, 0, 2)),
            "HEAD": np.ascontiguousarray(head),
            "QTF": np.ascontiguousarray(qtf).astype(np.float16),
            "MSW": np.ascontiguousarray(msw).astype(np.float16),
            "MS2": np.ascontiguousarray(ms2),
            "W1T": np.ascontiguousarray(
                w1T[:, msl].reshape(NCORES, 128, MSL).transpose(1, 0, 2)
            ).astype(np.float16),
            "FFNB": np.ascontiguousarray(b1f[msl].reshape(MCH, 128).T),
            "W1S": np.ascontiguousarray(w1sum[msl]).reshape(1, MSL),
            "W2T": np.ascontiguousarray(
                w2T[msl, :].reshape(MCH, 128, N).transpose(1, 0, 2)
            ).astype(np.float16),
        })

    trace = os.environ.get("BASS_KERNEL_TRACE", "0") == "1"
    res = run_bass_kernel_spmd(
        nc, in_maps, core_ids=list(range(NCORES)), trace=trace
    )
    last_results = res

    x = np.concatenate([res.results[c]["XT"] for c in range(NCORES)], axis=0).T
    h = np.zeros((N, B), dtype=np.float32)
    for c in range(NCORES):
        h += res.results[c]["HP"]
    out = x + h.T + ffn_b2[None, :]
    return out.astype(np.float32)


# revision 5
# speedup vs baseline: 1.0016x; 1.0016x over previous
"""Bass/Trainium2 kernel for nn_KernelizedAttentionResBlock.

Sharding: n-token sharded attention (each of 8 cores owns 128 rows of n for
all batches), one AllGather of x^T (n-major, fp16 payload), m-sharded FFN,
host sums the 8 partial FFN outputs and adds the residual/bias.

Levers:
- K/V + weights fp16: the kernel is DMA-bound at ~360 GB/s/core, so halving
  HBM traffic halves phase-2 time.
- Gaussian kernel in ONE activation pass: Derivative_Erf(rs*K - mu*rs) =
  2/sqrt(pi) * exp(-0.5 (K-mu)^2/(sigma^2+1e-8)); sqrt(pi)/2 folded into V
  on the host. (tensor_tensor_reduce would fuse the reduce too but wedges
  the device, so S*V and the D-reduce are separate ops.)
- The 32 multiplies/reduces are spread over DVE, Pool (gpsimd multiplies)
  and the Act engine (Identity with accum_out, which stays inside the
  erf_derivative act table), keeping every engine under the DMA pace.
- Only {Square, Tanh, Derivative_Erf, Sigmoid, Identity, Copy} activations:
  3 act-table loads, all off the critical path (a dummy Sigmoid pinned
  behind phase 2 preloads the FFN table during the collective). rsqrt is
  computed on the DVE via the 0x5f3759df bit trick + 1 Newton step (the
  Ln/Exp route would thrash activation tables).
- Both LayerNorms are folded into the adjacent matmuls in transposed
  layout: ln(v)^T @ W = rstd_b * (v^T @ W - mean_b * rowsum(W)); the
  rank-1 mean correction is an extra matmul accumulated into the same
  PSUM and rstd_b is a PE-broadcast row multiplied in afterwards (via an
  SBUF copy: tensor ops may read only one PSUM operand).
- Head tensors first on the SP queue so phase 0/1 starts immediately; FFN
  weights issued after cc_in so their transfer hides inside the collective.
"""
import os
import sys

sys.path.insert(0, "/opt/trn_rl_repo")

import numpy as np

N = 1024
B = 32
D = 1024
M = 4096
NCORES = 8
NSL = N // NCORES
MSL = M // NCORES
MCH = MSL // 128
NB = 4
LN_EPS = 1e-5
MAGIC1 = 0x5F3759E0  # 0x5f3759df + 1 (for MAGIC - x == ~x + MAGIC+1)
SQH = float(np.sqrt(0.5))

_built = {}
last_results = None


def _build_module():
    if "nc" in _built:
        return _built["nc"]

    import concourse.bacc as bacc
    import concourse.mybir as mybir
    import concourse.tile as tile

    AF = mybir.ActivationFunctionType
    ALU = mybir.AluOpType
    f32 = mybir.dt.float32
    f16 = mybir.dt.float16
    i32 = mybir.dt.int32

    nc = bacc.Bacc(trn_type="TRN2", num_devices=NCORES)

    Kd = nc.dram_tensor("Ks", (NSL, B, D), f16, kind="ExternalInput")
    Vd = nc.dram_tensor("Vs", (NSL, B, D), f16, kind="ExternalInput")
    # HEAD packs [-mu_b | sig_b | Q^T slice] along the free dim
    HEAD = nc.dram_tensor("HEAD", (128, 2 + B), f32, kind="ExternalInput")
    QTF = nc.dram_tensor("QTF", (128, NCORES, B), f16, kind="ExternalInput")
    MSW = nc.dram_tensor("MSW", (128, 2, NCORES, NSL), f16,
                         kind="ExternalInput")
    MS2 = nc.dram_tensor("MS2", (1, 2 * NSL), f32, kind="ExternalInput")
    W1T = nc.dram_tensor("W1T", (128, NCORES, MSL), f16, kind="ExternalInput")
    FFNB = nc.dram_tensor("FFNB", (128, MCH), f32, kind="ExternalInput")
    W1S = nc.dram_tensor("W1S", (1, MSL), f32, kind="ExternalInput")
    W2T = nc.dram_tensor("W2T", (128, MCH, N), f16, kind="ExternalInput")

    XTd = nc.dram_tensor("XT", (NSL, B), f32, kind="ExternalOutput")
    HPd = nc.dram_tensor("HP", (N, B), f32, kind="ExternalOutput")

    cc_in = nc.dram_tensor("cc_in", (NSL, B), f16, kind="Internal")
    cc_out = nc.dram_tensor(
        "cc_out", (N, B), f16, kind="Internal", addr_space="Shared"
    )

    def rsqrt_newton(pool, tag, v, steps, init, final_scale=1.0):
        """rsqrt(v) on the DVE: `init` is a same-shape f32 starting guess
        builder; 2 Newton steps; final_scale folded into the last step."""
        shape = list(v.shape)
        y = init
        for s in range(steps):
            k = final_scale if s == steps - 1 else 1.0
            t1 = pool.tile(shape, f32, tag=f"{tag}t1")
            nc.vector.tensor_mul(t1[:], y, y)
            t2 = pool.tile(shape, f32, tag=f"{tag}t2")
            nc.vector.tensor_mul(t2[:], t1[:], v[:])
            t3 = pool.tile(shape, f32, tag=f"{tag}t3")
            nc.vector.tensor_scalar(t3[:], t2[:], -0.5 * k, 1.5 * k,
                                    op0=ALU.mult, op1=ALU.add)
            yn = pool.tile(shape, f32, tag=f"{tag}n{s}")
            nc.vector.tensor_mul(yn[:], y, t3[:])
            y = yn[:]
        return y

    def rsqrt_bit(pool, tag, v, final_scale=1.0):
        """Full-range rsqrt: quake init + 2 Newton steps."""
        shape = list(v.shape)
        sh = pool.tile(shape, i32, tag=f"{tag}sh")
        nc.vector.tensor_scalar(sh[:], v[:].bitcast(i32), 1, None,
                                op0=ALU.logical_shift_right)
        y0i = pool.tile(shape, i32, tag=f"{tag}y0")
        nc.vector.tensor_scalar(y0i[:], sh[:], -1, MAGIC1 - 1,
                                op0=ALU.mult, op1=ALU.add)
        return rsqrt_newton(pool, tag, v, 1, y0i[:].bitcast(f32),
                            final_scale)

    with tile.TileContext(nc) as tc:
        with tc.tile_pool(name="const", bufs=1) as cst, \
             tc.tile_pool(name="small", bufs=1) as sm, \
             tc.tile_pool(name="kv", bufs=6) as kv, \
             tc.tile_pool(name="scr", bufs=4) as scr, \
             tc.tile_pool(name="psum", bufs=1, space="PSUM") as ps:

            # ---- head tensors first, then the K/V stream (SP queue) ---
            head = cst.tile([128, 2 + B], f32)
            nc.sync.dma_start(head[:], HEAD[:])
            qtf = cst.tile([128, NCORES, B], f16)
            nc.sync.dma_start(qtf[:], QTF[:])
            msw = cst.tile([128, 2, NCORES, NSL], f16)
            nc.sync.dma_start(msw[:], MSW[:])
            ms2 = cst.tile([1, 2 * NSL], f32)
            nc.sync.dma_start(ms2[:], MS2[:])

            BLOCKS = [(i * NB, NB) for i in range(B // NB - 1)]
            BLOCKS += [(B - NB, NB // 2), (B - NB // 2, NB // 2)]
            kts, vts = [], []
            for b0, nb in BLOCKS:
                kt = kv.tile([NSL, nb, D], f16, tag=f"kt{nb}")
                nc.sync.dma_start(kt[:], Kd[:, b0:b0 + nb, :])
                vt = kv.tile([NSL, nb, D], f16, tag=f"vt{nb}")
                nc.sync.dma_start(vt[:], Vd[:, b0:b0 + nb, :])
                kts.append(kt)
                vts.append(vt)

            # ---- head tensors on the Pool queue -----------------------
            nmb = head[:, 0:1]
            sb1 = head[:, 1:2]
            qts = head[:, 2:2 + B]

            ones16 = cst.tile([128, 1], f16)
            nc.vector.memset(ones16[:], 1.0)
            ones32 = cst.tile([128, 1], f32)
            nc.vector.memset(ones32[:], 1.0)
            ones_row = cst.tile([1, 128], f32)
            nc.vector.memset(ones_row[:], 1.0)

            # ---------- Phase 0/1: LN(Q) folded into mu/sigma -----------
            qsq = sm.tile([128, NCORES, B], f32)
            nc.scalar.activation(qsq[:], qtf[:], AF.Square)
            qs_ps = ps.tile([1, B], f32, tag="pA")
            for c in range(NCORES):
                nc.tensor.matmul(qs_ps[:], ones16[:], qtf[:, c, :],
                                 start=(c == 0), stop=(c == NCORES - 1))
            qs2_ps = ps.tile([1, B], f32, tag="pB")
            for c in range(NCORES):
                nc.tensor.matmul(qs2_ps[:], ones32[:], qsq[:, c, :],
                                 start=(c == 0), stop=(c == NCORES - 1))
            negmean = sm.tile([1, B], f32)
            nc.vector.tensor_scalar_mul(negmean[:], qs_ps[:], -1.0 / N)
            msq = sm.tile([1, B], f32)
            nc.vector.tensor_mul(msq[:], negmean[:], negmean[:])
            varq = sm.tile([1, B], f32)
            nc.vector.tensor_scalar(varq[:], qs2_ps[:], 1.0 / N, LN_EPS,
                                    op0=ALU.mult, op1=ALU.add)
            varq2 = sm.tile([1, B], f32)
            nc.vector.tensor_sub(varq2[:], varq[:], msq[:])
            # q-row variance is ~1 (Q ~ N(0,1)): linear init + 2 Newton
            y0q = sm.tile([1, B], f32)
            nc.vector.tensor_scalar(y0q[:], varq2[:], -0.5, 1.5,
                                    op0=ALU.mult, op1=ALU.add)
            rstdq = rsqrt_newton(sm, "rq", varq2, 1, y0q[:])
            RSTD0 = ps.tile([128, B], f32, tag="pC")
            nc.tensor.matmul(RSTD0[:], ones_row[:], rstdq,
                             start=True, stop=True)
            rstd0_sb = sm.tile([128, B], f32)
            nc.vector.tensor_scalar_mul(rstd0_sb[:], RSTD0[:], 1.0)

            mu_ps = ps.tile([NSL, B], f32, tag="pA")
            nc.tensor.matmul(mu_ps[:], ms2[:, 0:NSL], negmean[:],
                             start=True, stop=False)
            for c in range(NCORES):
                nc.tensor.matmul(mu_ps[:], msw[:, 0, c, :], qtf[:, c, :],
                                 start=False, stop=(c == NCORES - 1))
            zmu = sm.tile([NSL, B], f32)
            nc.vector.tensor_mul(zmu[:], mu_ps[:], rstd0_sb[:])
            negmu = sm.tile([NSL, B], f32)
            nc.scalar.activation(negmu[:], zmu[:], AF.Tanh,
                                 bias=nmb, scale=-1.0)

            sig_ps = ps.tile([NSL, B], f32, tag="pB")
            nc.tensor.matmul(sig_ps[:], ms2[:, NSL:2 * NSL], negmean[:],
                             start=True, stop=False)
            for c in range(NCORES):
                nc.tensor.matmul(sig_ps[:], msw[:, 1, c, :], qtf[:, c, :],
                                 start=False, stop=(c == NCORES - 1))
            zsig = sm.tile([NSL, B], f32)
            nc.vector.tensor_mul(zsig[:], sig_ps[:], rstd0_sb[:])
            s2 = sm.tile([NSL, B], f32)
            nc.scalar.activation(s2[:], zsig[:], AF.Square, bias=sb1)
            s2e = sm.tile([NSL, B], f32)
            nc.vector.tensor_scalar_add(s2e[:], s2[:], 1e-8)
            # rs = sqrt(0.5/(sigma^2+1e-8)) — full-range bit-trick rsqrt
            rs = rsqrt_bit(sm, "rs", s2e, final_scale=SQH)
            nmr = sm.tile([NSL, B], f32)
            nc.vector.tensor_mul(nmr[:], negmu[:], rs)

            # ---------- Phase 2: x^T = sum_D S*V' + Q^T -----------------
            # tensor_tensor_reduce wedges the device; use mult + reduce,
            # spreading 12 of 32 reduces onto the idle Pool engine.
            AX = mybir.AxisListType
            xT = sm.tile([NSL, B], f32)
            for blk, (b0, nb) in enumerate(BLOCKS):
                kt, vt = kts[blk], vts[blk]
                for bi in range(nb):
                    b = b0 + bi
                    es = scr.tile([NSL, D], f16, tag="es")
                    nc.scalar.activation(es[:], kt[:, bi, :],
                                         AF.Derivative_Erf,
                                         bias=nmr[:, b:b + 1],
                                         scale=rs[:, b:b + 1])
                    sv = scr.tile([NSL, D], f16, tag="sv")
                    meng = nc.gpsimd if b % 4 == 1 else nc.vector
                    meng.tensor_mul(sv[:], es[:], vt[:, bi, :])
                    if b % 4 == 3:
                        # Act engine reduce: Identity with accumulator out
                        # (identity is in the erf_derivative table: no load)
                        ad = scr.tile([NSL, D], f16, tag="ad")
                        nc.scalar.activation(ad[:], sv[:], AF.Identity,
                                             accum_out=xT[:, b:b + 1])
                    else:
                        nc.vector.reduce_sum(xT[:, b:b + 1], sv[:],
                                             axis=AX.X)
            # x = A + Q fused with the f16 cast; XTd ships raw A and the
            # host adds the Q residual in f32.
            xh16 = sm.tile([NSL, B], f16)
            nc.vector.tensor_add(xh16[:], xT[:], qts)
            nc.sync.dma_start(cc_in[:], xh16[:])
            nc.sync.dma_start(XTd[:], xT[:])

            # preload the sigmoid act table during the collective window
            # (input xh16 pins it after phase 2 so the load can't be hoisted)
            sgd = sm.tile([NSL, 1], f32)
            nc.scalar.activation(sgd[:], xh16[:, 0:1], AF.Sigmoid)

            # FFN weights: transfers land inside the collective window.
            w1T = cst.tile([128, NCORES, MSL], f16)
            nc.sync.dma_start(w1T[:], W1T[:])
            ffnb = cst.tile([128, MCH], f32)
            nc.sync.dma_start(ffnb[:], FFNB[:])
            w1s = cst.tile([1, MSL], f32)
            nc.sync.dma_start(w1s[:], W1S[:])
            w2T = cst.tile([128, MCH, N], f16)
            nc.sync.dma_start(w2T[:], W2T[:])

            # ---------- Phase 3: AllGather x^T (n-major), LN, FFN -------
            nc.gpsimd.collective_compute(
                "AllGather", ALU.bypass,
                replica_groups=[list(range(NCORES))],
                ins=[cc_in[:]], outs=[cc_out[:]],
            )
            xg16 = sm.tile([128, NCORES, B], f16)
            nc.sync.dma_start(
                xg16[:], cc_out[:].rearrange("(c p) b -> p c b", p=128))
            xsq = sm.tile([128, NCORES, B], f32)
            nc.vector.tensor_mul(xsq[:], xg16[:], xg16[:])
            s_ps = ps.tile([1, B], f32, tag="pA")
            for c in range(NCORES):
                nc.tensor.matmul(s_ps[:], ones16[:], xg16[:, c, :],
                                 start=(c == 0), stop=(c == NCORES - 1))
            s2_ps = ps.tile([1, B], f32, tag="pB")
            for c in range(NCORES):
                nc.tensor.matmul(s2_ps[:], ones32[:], xsq[:, c, :],
                                 start=(c == 0), stop=(c == NCORES - 1))
            negmx = sm.tile([1, B], f32)
            nc.vector.tensor_scalar_mul(negmx[:], s_ps[:], -1.0 / N)
            msqx = sm.tile([1, B], f32)
            nc.vector.tensor_mul(msqx[:], negmx[:], negmx[:])
            varx = sm.tile([1, B], f32)
            nc.vector.tensor_scalar(varx[:], s2_ps[:], 1.0 / N, LN_EPS,
                                    op0=ALU.mult, op1=ALU.add)
            varx2 = sm.tile([1, B], f32)
            nc.vector.tensor_sub(varx2[:], varx[:], msqx[:])
            rstdx = rsqrt_bit(sm, "rx", varx2)
            RSTD1 = ps.tile([128, B], f32, tag="pC")
            nc.tensor.matmul(RSTD1[:], ones_row[:], rstdx,
                             start=True, stop=True)
            rstd1_sb = sm.tile([128, B], f32)
            nc.vector.tensor_scalar_mul(rstd1_sb[:], RSTD1[:], 1.0)

            # FFN: h1 = (x@w1 - mean*w1sum)*rstd + b1 ; silu = z*sigmoid(z)
            g1_sb = sm.tile([128, MCH, B], f16)
            for mi in range(MCH):
                h1_ps = ps.tile([128, B], f32, tag=f"p{chr(68 + mi)}")
                nc.tensor.matmul(h1_ps[:], w1s[:, mi * 128:(mi + 1) * 128],
                                 negmx[:], start=True, stop=False)
                for c in range(NCORES):
                    nc.tensor.matmul(h1_ps[:],
                                     w1T[:, c, mi * 128:(mi + 1) * 128],
                                     xg16[:, c, :],
                                     start=False, stop=(c == NCORES - 1))
                zpre = sm.tile([128, B], f32, tag=f"zp_{mi}")
                nc.vector.tensor_mul(zpre[:], h1_ps[:], rstd1_sb[:])
                sg = sm.tile([128, B], f32, tag=f"sg_{mi}")
                nc.scalar.activation(sg[:], zpre[:], AF.Sigmoid,
                                     bias=ffnb[:, mi:mi + 1])
                z = sm.tile([128, B], f32, tag=f"z_{mi}")
                nc.vector.tensor_scalar_add(z[:], zpre[:],
                                            ffnb[:, mi:mi + 1])
                nc.vector.tensor_mul(g1_sb[:, mi, :], z[:], sg[:])

            hp_sb = sm.tile([128, NCORES, B], f32)
            hpv = HPd[:].rearrange("(jn p) b -> p jn b", p=128)
            for jn in range(NCORES):
                hp_ps = ps.tile([128, B], f32, tag=f"p{chr(68 + jn % 4)}")
                for mi in range(MCH):
                    nc.tensor.matmul(hp_ps[:],
                                     w2T[:, mi, jn * 128:(jn + 1) * 128],
                                     g1_sb[:, mi, :],
                                     start=(mi == 0), stop=(mi == MCH - 1))
                if jn % 2 == 0:
                    nc.scalar.copy(hp_sb[:, jn, :], hp_ps[:])
                else:
                    nc.vector.tensor_scalar_mul(hp_sb[:, jn, :], hp_ps[:],
                                                1.0)
                if jn == 3:
                    nc.sync.dma_start(hpv[:, 0:4, :], hp_sb[:, 0:4, :])
            nc.sync.dma_start(hpv[:, 4:8, :], hp_sb[:, 4:8, :])

    nc.finalize()
    _built["nc"] = nc
    return nc


def kernel(**inputs):
    from concourse.bass_utils import run_bass_kernel_spmd

    global last_results

    Q = np.asarray(inputs["Q"], dtype=np.float32)
    K = np.asarray(inputs["K"], dtype=np.float32)
    V = np.asarray(inputs["V"], dtype=np.float32)
    mu_w = np.asarray(inputs["mu_w"], dtype=np.float32)
    mu_b = np.asarray(inputs["mu_b"], dtype=np.float32)
    sigma_w = np.asarray(inputs["sigma_w"], dtype=np.float32)
    sigma_b = np.asarray(inputs["sigma_b"], dtype=np.float32)
    ffn_w1 = np.asarray(inputs["ffn_w1"], dtype=np.float32)
    ffn_b1 = np.asarray(inputs["ffn_b1"], dtype=np.float32)
    ffn_w2 = np.asarray(inputs["ffn_w2"], dtype=np.float32)
    ffn_b2 = np.asarray(inputs["ffn_b2"], dtype=np.float32)
    ln_ff_g = np.asarray(inputs["ln_ff_g"], dtype=np.float32)
    ln_ff_b = np.asarray(inputs["ln_ff_b"], dtype=np.float32)
    ln_q_g = np.asarray(inputs["ln_q_g"], dtype=np.float32)
    ln_q_b = np.asarray(inputs["ln_q_b"], dtype=np.float32)

    # ---- Host-side exact folds of LN affine params into next matmuls ----
    mu_wf = mu_w * ln_q_g[None, :]
    mu_bf = mu_b + mu_w @ ln_q_b
    sig_wf = sigma_w * ln_q_g[None, :]
    sig_bf = sigma_b + sigma_w @ ln_q_b
    w1f = ffn_w1 * ln_ff_g[None, :]
    b1f = ffn_b1 + ffn_w1 @ ln_ff_b
    w1sum = w1f.sum(axis=1)
    musum = mu_wf.sum(axis=1)
    sigsum = sig_wf.sum(axis=1)

    # Device computes S*V' with S = Derivative_Erf(u) = 2/sqrt(pi)*exp(-u^2)
    Vs = (V * (np.sqrt(np.pi) / 2.0)).astype(np.float16)
    Kh = K.astype(np.float16)

    QT = np.ascontiguousarray(Q.T)                    # (N, B)
    qtf = QT.reshape(NCORES, 128, B).transpose(1, 0, 2)
    muwT = np.ascontiguousarray(mu_wf.T)              # (N, N)  [jn, j]
    sigwT = np.ascontiguousarray(sig_wf.T)
    w1T = np.ascontiguousarray(w1f.T)                 # (N, M)
    w2T = np.ascontiguousarray(ffn_w2.T)              # (M, N)

    nc = _build_module()

    in_maps = []
    for c in range(NCORES):
        jsl = slice(c * NSL, (c + 1) * NSL)
        msl = slice(c * MSL, (c + 1) * MSL)
        head = np.concatenate([
            (-mu_bf[jsl]).reshape(NSL, 1),
            sig_bf[jsl].reshape(NSL, 1),
            QT[jsl, :],
        ], axis=1)
        msw = np.stack([
            muwT[:, jsl].reshape(NCORES, 128, NSL).transpose(1, 0, 2),
            sigwT[:, jsl].reshape(NCORES, 128, NSL).transpose(1, 0, 2),
        ], axis=1)                                    # (128, 2, 8, NSL)
        ms2 = np.concatenate([musum[jsl], sigsum[jsl]]).reshape(1, 2 * NSL)
        in_maps.append({
            "Ks": np.ascontiguousarray(Kh[:, jsl, :].transpose(1, 0, 2)),
            "Vs": np.ascontiguousarray(Vs[:, jsl, :].transpose(1, 0, 2)),
            "HEAD": np.ascontiguousarray(head),
            "QTF": np.ascontiguousarray(qtf).astype(np.float16),
            "MSW": np.ascontiguousarray(msw).astype(np.float16),
            "MS2": np.ascontiguousarray(ms2),
            "W1T": np.ascontiguousarray(
                w1T[:, msl].reshape(NCORES, 128, MSL).transpose(1, 0, 2)
            ).astype(np.float16),
            "FFNB": np.ascontiguousarray(b1f[msl].reshape(MCH, 128).T),
            "W1S": np.ascontiguousarray(w1sum[msl]).reshape(1, MSL),
            "W2T": np.ascontiguousarray(
                w2T[msl, :].reshape(MCH, 128, N).transpose(1, 0, 2)
            ).astype(np.float16),
        })

    trace = os.environ.get("BASS_KERNEL_TRACE", "0") == "1"
    res = run_bass_kernel_spmd(
        nc, in_maps, core_ids=list(range(NCORES)), trace=trace
    )
    last_results = res

    x = np.concatenate([res.results[c]["XT"] for c in range(NCORES)],
                       axis=0).T + Q
    h = np.zeros((N, B), dtype=np.float32)
    for c in range(NCORES):
        h += res.results[c]["HP"]
    out = x + h.T + ffn_b2[None, :]
    return out.astype(np.float32)


# revision 6
# speedup vs baseline: 1.0075x; 1.0059x over previous
"""Bass/Trainium2 kernel for nn_KernelizedAttentionResBlock.

Sharding: n-token sharded attention (each of 8 cores owns 128 rows of n for
all batches), one AllGather of x^T (n-major, fp16 payload), m-sharded FFN,
host sums the 8 partial FFN outputs and adds the residual/bias.

Levers:
- K/V + weights fp16: the kernel is DMA-bound at ~360 GB/s/core, so halving
  HBM traffic halves phase-2 time.
- Gaussian kernel in ONE activation pass: Derivative_Erf(rs*K - mu*rs) =
  2/sqrt(pi) * exp(-0.5 (K-mu)^2/(sigma^2+1e-8)); sqrt(pi)/2 folded into V
  on the host. (tensor_tensor_reduce would fuse the reduce too but wedges
  the device, so S*V and the D-reduce are separate ops.)
- The 32 multiplies/reduces are spread over DVE, Pool (gpsimd multiplies)
  and the Act engine (Identity with accum_out, which stays inside the
  erf_derivative act table), keeping every engine under the DMA pace.
- Only {Square, Tanh, Derivative_Erf, Sigmoid, Identity, Copy} activations:
  3 act-table loads, all off the critical path (a dummy Sigmoid pinned
  behind phase 2 preloads the FFN table during the collective). rsqrt is
  computed on the DVE via the 0x5f3759df bit trick + 1 Newton step (the
  Ln/Exp route would thrash activation tables).
- Both LayerNorms are folded into the adjacent matmuls in transposed
  layout: ln(v)^T @ W = rstd_b * (v^T @ W - mean_b * rowsum(W)); the
  rank-1 mean correction is an extra matmul accumulated into the same
  PSUM and rstd_b is a PE-broadcast row multiplied in afterwards (via an
  SBUF copy: tensor ops may read only one PSUM operand).
- Head tensors first on the SP queue so phase 0/1 starts immediately; FFN
  weights issued after cc_in so their transfer hides inside the collective.
"""
import os
import sys

sys.path.insert(0, "/opt/trn_rl_repo")

import numpy as np

N = 1024
B = 32
D = 1024
M = 4096
NCORES = 8
NSL = N // NCORES
MSL = M // NCORES
MCH = MSL // 128
NB = 4
LN_EPS = 1e-5
MAGIC1 = 0x5F3759E0  # 0x5f3759df + 1 (for MAGIC - x == ~x + MAGIC+1)
SQH = float(np.sqrt(0.5))

_built = {}
last_results = None


def _build_module():
    if "nc" in _built:
        return _built["nc"]

    import concourse.bacc as bacc
    import concourse.mybir as mybir
    import concourse.tile as tile

    AF = mybir.ActivationFunctionType
    ALU = mybir.AluOpType
    f32 = mybir.dt.float32
    f16 = mybir.dt.float16
    i32 = mybir.dt.int32
    f8 = mybir.dt.float8e4

    nc = bacc.Bacc(trn_type="TRN2", num_devices=NCORES)

    Kd = nc.dram_tensor("Ks", (NSL, B, D), f16, kind="ExternalInput")
    Vd = nc.dram_tensor("Vs", (NSL, B, D), f16, kind="ExternalInput")
    # HEAD packs [-mu_b | sig_b | Q^T slice] along the free dim
    HEAD = nc.dram_tensor("HEAD", (128, 2 + B), f32, kind="ExternalInput")
    QTF = nc.dram_tensor("QTF", (128, NCORES, B), f16, kind="ExternalInput")
    MSW = nc.dram_tensor("MSW", (128, 2, NCORES, NSL), f16,
                         kind="ExternalInput")
    MS2 = nc.dram_tensor("MS2", (1, 2 * NSL), f32, kind="ExternalInput")
    W1T = nc.dram_tensor("W1T", (128, NCORES, MSL), f16, kind="ExternalInput")
    FFNB = nc.dram_tensor("FFNB", (128, MCH), f32, kind="ExternalInput")
    W1S = nc.dram_tensor("W1S", (1, MSL), f32, kind="ExternalInput")
    W2T = nc.dram_tensor("W2T", (128, MCH, N), f16, kind="ExternalInput")

    XTd = nc.dram_tensor("XT", (NSL, B), f32, kind="ExternalOutput")
    HPd = nc.dram_tensor("HP", (N, B), f32, kind="ExternalOutput")

    cc_in = nc.dram_tensor("cc_in", (NSL, B), f8, kind="Internal")
    cc_out = nc.dram_tensor(
        "cc_out", (N, B), f8, kind="Internal", addr_space="Shared"
    )

    def rsqrt_newton(pool, tag, v, steps, init, final_scale=1.0):
        """rsqrt(v) on the DVE: `init` is a same-shape f32 starting guess
        builder; 2 Newton steps; final_scale folded into the last step."""
        shape = list(v.shape)
        y = init
        for s in range(steps):
            k = final_scale if s == steps - 1 else 1.0
            t1 = pool.tile(shape, f32, tag=f"{tag}t1")
            nc.vector.tensor_mul(t1[:], y, y)
            t2 = pool.tile(shape, f32, tag=f"{tag}t2")
            nc.vector.tensor_mul(t2[:], t1[:], v[:])
            t3 = pool.tile(shape, f32, tag=f"{tag}t3")
            nc.vector.tensor_scalar(t3[:], t2[:], -0.5 * k, 1.5 * k,
                                    op0=ALU.mult, op1=ALU.add)
            yn = pool.tile(shape, f32, tag=f"{tag}n{s}")
            nc.vector.tensor_mul(yn[:], y, t3[:])
            y = yn[:]
        return y

    def rsqrt_bit(pool, tag, v, final_scale=1.0):
        """Full-range rsqrt: quake init + 2 Newton steps."""
        shape = list(v.shape)
        sh = pool.tile(shape, i32, tag=f"{tag}sh")
        nc.vector.tensor_scalar(sh[:], v[:].bitcast(i32), 1, None,
                                op0=ALU.logical_shift_right)
        y0i = pool.tile(shape, i32, tag=f"{tag}y0")
        nc.vector.tensor_scalar(y0i[:], sh[:], -1, MAGIC1 - 1,
                                op0=ALU.mult, op1=ALU.add)
        return rsqrt_newton(pool, tag, v, 1, y0i[:].bitcast(f32),
                            final_scale)

    with tile.TileContext(nc) as tc:
        with tc.tile_pool(name="const", bufs=1) as cst, \
             tc.tile_pool(name="small", bufs=1) as sm, \
             tc.tile_pool(name="kv", bufs=6) as kv, \
             tc.tile_pool(name="scr", bufs=4) as scr, \
             tc.tile_pool(name="psum", bufs=1, space="PSUM") as ps:

            # ---- head tensors first, then the K/V stream (SP queue) ---
            head = cst.tile([128, 2 + B], f32)
            nc.sync.dma_start(head[:], HEAD[:])
            qtf = cst.tile([128, NCORES, B], f16)
            nc.sync.dma_start(qtf[:], QTF[:])
            msw = cst.tile([128, 2, NCORES, NSL], f16)
            nc.sync.dma_start(msw[:], MSW[:])
            ms2 = cst.tile([1, 2 * NSL], f32)
            nc.sync.dma_start(ms2[:], MS2[:])

            BLOCKS = [(i * NB, NB) for i in range(B // NB - 1)]
            BLOCKS += [(B - NB, NB // 2), (B - NB // 2, NB // 2)]
            kts, vts = [], []
            for b0, nb in BLOCKS:
                kt = kv.tile([NSL, nb, D], f16, tag=f"kt{nb}")
                nc.sync.dma_start(kt[:], Kd[:, b0:b0 + nb, :])
                vt = kv.tile([NSL, nb, D], f16, tag=f"vt{nb}")
                nc.sync.dma_start(vt[:], Vd[:, b0:b0 + nb, :])
                kts.append(kt)
                vts.append(vt)

            # ---- head tensors on the Pool queue -----------------------
            nmb = head[:, 0:1]
            sb1 = head[:, 1:2]
            qts = head[:, 2:2 + B]

            ones16 = cst.tile([128, 1], f16)
            nc.vector.memset(ones16[:], 1.0)
            ones32 = cst.tile([128, 1], f32)
            nc.vector.memset(ones32[:], 1.0)
            ones_row = cst.tile([1, 128], f32)
            nc.vector.memset(ones_row[:], 1.0)

            # ---------- Phase 0/1: LN(Q) folded into mu/sigma -----------
            qsq = sm.tile([128, NCORES, B], f32)
            nc.scalar.activation(qsq[:], qtf[:], AF.Square)
            qs_ps = ps.tile([1, B], f32, tag="pA")
            for c in range(NCORES):
                nc.tensor.matmul(qs_ps[:], ones16[:], qtf[:, c, :],
                                 start=(c == 0), stop=(c == NCORES - 1))
            qs2_ps = ps.tile([1, B], f32, tag="pB")
            for c in range(NCORES):
                nc.tensor.matmul(qs2_ps[:], ones32[:], qsq[:, c, :],
                                 start=(c == 0), stop=(c == NCORES - 1))
            negmean = sm.tile([1, B], f32)
            nc.vector.tensor_scalar_mul(negmean[:], qs_ps[:], -1.0 / N)
            msq = sm.tile([1, B], f32)
            nc.vector.tensor_mul(msq[:], negmean[:], negmean[:])
            varq = sm.tile([1, B], f32)
            nc.vector.tensor_scalar(varq[:], qs2_ps[:], 1.0 / N, LN_EPS,
                                    op0=ALU.mult, op1=ALU.add)
            varq2 = sm.tile([1, B], f32)
            nc.vector.tensor_sub(varq2[:], varq[:], msq[:])
            # q-row variance is ~1 (Q ~ N(0,1)): linear init + 2 Newton
            y0q = sm.tile([1, B], f32)
            nc.vector.tensor_scalar(y0q[:], varq2[:], -0.5, 1.5,
                                    op0=ALU.mult, op1=ALU.add)
            rstdq = rsqrt_newton(sm, "rq", varq2, 1, y0q[:])
            RSTD0 = ps.tile([128, B], f32, tag="pC")
            nc.tensor.matmul(RSTD0[:], ones_row[:], rstdq,
                             start=True, stop=True)
            rstd0_sb = sm.tile([128, B], f32)
            nc.vector.tensor_scalar_mul(rstd0_sb[:], RSTD0[:], 1.0)

            mu_ps = ps.tile([NSL, B], f32, tag="pA")
            nc.tensor.matmul(mu_ps[:], ms2[:, 0:NSL], negmean[:],
                             start=True, stop=False)
            for c in range(NCORES):
                nc.tensor.matmul(mu_ps[:], msw[:, 0, c, :], qtf[:, c, :],
                                 start=False, stop=(c == NCORES - 1))
            zmu = sm.tile([NSL, B], f32)
            nc.vector.tensor_mul(zmu[:], mu_ps[:], rstd0_sb[:])
            negmu = sm.tile([NSL, B], f32)
            nc.scalar.activation(negmu[:], zmu[:], AF.Tanh,
                                 bias=nmb, scale=-1.0)

            sig_ps = ps.tile([NSL, B], f32, tag="pB")
            nc.tensor.matmul(sig_ps[:], ms2[:, NSL:2 * NSL], negmean[:],
                             start=True, stop=False)
            for c in range(NCORES):
                nc.tensor.matmul(sig_ps[:], msw[:, 1, c, :], qtf[:, c, :],
                                 start=False, stop=(c == NCORES - 1))
            zsig = sm.tile([NSL, B], f32)
            nc.vector.tensor_mul(zsig[:], sig_ps[:], rstd0_sb[:])
            s2 = sm.tile([NSL, B], f32)
            nc.scalar.activation(s2[:], zsig[:], AF.Square, bias=sb1)
            s2e = sm.tile([NSL, B], f32)
            nc.vector.tensor_scalar_add(s2e[:], s2[:], 1e-8)
            # rs = sqrt(0.5/(sigma^2+1e-8)) — full-range bit-trick rsqrt
            rs = rsqrt_bit(sm, "rs", s2e, final_scale=SQH)
            nmr = sm.tile([NSL, B], f32)
            nc.vector.tensor_mul(nmr[:], negmu[:], rs)

            # ---------- Phase 2: x^T = sum_D S*V' + Q^T -----------------
            # tensor_tensor_reduce wedges the device; use mult + reduce,
            # spreading 12 of 32 reduces onto the idle Pool engine.
            AX = mybir.AxisListType
            xT = sm.tile([NSL, B], f32)
            for blk, (b0, nb) in enumerate(BLOCKS):
                kt, vt = kts[blk], vts[blk]
                for bi in range(nb):
                    b = b0 + bi
                    es = scr.tile([NSL, D], f16, tag="es")
                    nc.scalar.activation(es[:], kt[:, bi, :],
                                         AF.Derivative_Erf,
                                         bias=nmr[:, b:b + 1],
                                         scale=rs[:, b:b + 1])
                    sv = scr.tile([NSL, D], f16, tag="sv")
                    meng = nc.gpsimd if b % 4 == 1 else nc.vector
                    meng.tensor_mul(sv[:], es[:], vt[:, bi, :])
                    if b % 4 == 3:
                        # Act engine reduce: Identity with accumulator out
                        # (identity is in the erf_derivative table: no load)
                        ad = scr.tile([NSL, D], f16, tag="ad")
                        nc.scalar.activation(ad[:], sv[:], AF.Identity,
                                             accum_out=xT[:, b:b + 1])
                    else:
                        nc.vector.reduce_sum(xT[:, b:b + 1], sv[:],
                                             axis=AX.X)
            # x = A + Q fused with the f16 cast; XTd ships raw A and the
            # host adds the Q residual in f32.
            xh16 = sm.tile([NSL, B], f8)
            nc.vector.tensor_add(xh16[:], xT[:], qts)
            nc.sync.dma_start(cc_in[:], xh16[:])
            nc.sync.dma_start(XTd[:], xT[:])

            # preload the sigmoid act table during the collective window
            # (input xh16 pins it after phase 2 so the load can't be hoisted)
            sgd = sm.tile([NSL, 1], f32)
            nc.scalar.activation(sgd[:], xh16[:, 0:1], AF.Sigmoid)

            # FFN weights: transfers land inside the collective window.
            w1T = cst.tile([128, NCORES, MSL], f16)
            nc.sync.dma_start(w1T[:], W1T[:])
            ffnb = cst.tile([128, MCH], f32)
            nc.sync.dma_start(ffnb[:], FFNB[:])
            w1s = cst.tile([1, MSL], f32)
            nc.sync.dma_start(w1s[:], W1S[:])
            w2T = cst.tile([128, MCH, N], f16)
            nc.sync.dma_start(w2T[:], W2T[:])

            # ---------- Phase 3: AllGather x^T (n-major), LN, FFN -------
            nc.gpsimd.collective_compute(
                "AllGather", ALU.bypass,
                replica_groups=[list(range(NCORES))],
                ins=[cc_in[:]], outs=[cc_out[:]],
            )
            xg8 = sm.tile([128, NCORES, B], f8)
            nc.sync.dma_start(
                xg8[:], cc_out[:].rearrange("(c p) b -> p c b", p=128))
            xg16 = sm.tile([128, NCORES, B], f16)
            nc.vector.tensor_scalar_mul(xg16[:], xg8[:], 1.0)
            xsq = sm.tile([128, NCORES, B], f32)
            nc.vector.tensor_mul(xsq[:], xg16[:], xg16[:])
            s_ps = ps.tile([1, B], f32, tag="pA")
            for c in range(NCORES):
                nc.tensor.matmul(s_ps[:], ones16[:], xg16[:, c, :],
                                 start=(c == 0), stop=(c == NCORES - 1))
            s2_ps = ps.tile([1, B], f32, tag="pB")
            for c in range(NCORES):
                nc.tensor.matmul(s2_ps[:], ones32[:], xsq[:, c, :],
                                 start=(c == 0), stop=(c == NCORES - 1))
            negmx = sm.tile([1, B], f32)
            nc.vector.tensor_scalar_mul(negmx[:], s_ps[:], -1.0 / N)
            msqx = sm.tile([1, B], f32)
            nc.vector.tensor_mul(msqx[:], negmx[:], negmx[:])
            varx = sm.tile([1, B], f32)
            nc.vector.tensor_scalar(varx[:], s2_ps[:], 1.0 / N, LN_EPS,
                                    op0=ALU.mult, op1=ALU.add)
            varx2 = sm.tile([1, B], f32)
            nc.vector.tensor_sub(varx2[:], varx[:], msqx[:])
            rstdx = rsqrt_bit(sm, "rx", varx2)
            RSTD1 = ps.tile([128, B], f32, tag="pC")
            nc.tensor.matmul(RSTD1[:], ones_row[:], rstdx,
                             start=True, stop=True)
            rstd1_sb = sm.tile([128, B], f32)
            nc.vector.tensor_scalar_mul(rstd1_sb[:], RSTD1[:], 1.0)

            # FFN: h1 = (x@w1 - mean*w1sum)*rstd + b1 ; silu = z*sigmoid(z)
            g1_sb = sm.tile([128, MCH, B], f16)
            for mi in range(MCH):
                h1_ps = ps.tile([128, B], f32, tag=f"p{chr(68 + mi)}")
                nc.tensor.matmul(h1_ps[:], w1s[:, mi * 128:(mi + 1) * 128],
                                 negmx[:], start=True, stop=False)
                for c in range(NCORES):
                    nc.tensor.matmul(h1_ps[:],
                                     w1T[:, c, mi * 128:(mi + 1) * 128],
                                     xg16[:, c, :],
                                     start=False, stop=(c == NCORES - 1))
                zpre = sm.tile([128, B], f32, tag=f"zp_{mi}")
                nc.vector.tensor_mul(zpre[:], h1_ps[:], rstd1_sb[:])
                sg = sm.tile([128, B], f32, tag=f"sg_{mi}")
                nc.scalar.activation(sg[:], zpre[:], AF.Sigmoid,
                                     bias=ffnb[:, mi:mi + 1])
                z = sm.tile([128, B], f32, tag=f"z_{mi}")
                nc.vector.tensor_scalar_add(z[:], zpre[:],
                                            ffnb[:, mi:mi + 1])
                nc.vector.tensor_mul(g1_sb[:, mi, :], z[:], sg[:])

            hp_sb = sm.tile([128, NCORES, B], f32)
            hpv = HPd[:].rearrange("(jn p) b -> p jn b", p=128)
            for jn in range(NCORES):
                hp_ps = ps.tile([128, B], f32, tag=f"p{chr(68 + jn % 4)}")
                for mi in range(MCH):
                    nc.tensor.matmul(hp_ps[:],
                                     w2T[:, mi, jn * 128:(jn + 1) * 128],
                                     g1_sb[:, mi, :],
                                     start=(mi == 0), stop=(mi == MCH - 1))
                if jn % 2 == 0:
                    nc.scalar.copy(hp_sb[:, jn, :], hp_ps[:])
                else:
                    nc.vector.tensor_scalar_mul(hp_sb[:, jn, :], hp_ps[:],
                                                1.0)
                if jn == 3:
                    nc.sync.dma_start(hpv[:, 0:4, :], hp_sb[:, 0:4, :])
            nc.sync.dma_start(hpv[:, 4:8, :], hp_sb[:, 4:8, :])

    nc.finalize()
    _built["nc"] = nc
    return nc


def kernel(**inputs):
    from concourse.bass_utils import run_bass_kernel_spmd

    global last_results

    Q = np.asarray(inputs["Q"], dtype=np.float32)
    K = np.asarray(inputs["K"], dtype=np.float32)
    V = np.asarray(inputs["V"], dtype=np.float32)
    mu_w = np.asarray(inputs["mu_w"], dtype=np.float32)
    mu_b = np.asarray(inputs["mu_b"], dtype=np.float32)
    sigma_w = np.asarray(inputs["sigma_w"], dtype=np.float32)
    sigma_b = np.asarray(inputs["sigma_b"], dtype=np.float32)
    ffn_w1 = np.asarray(inputs["ffn_w1"], dtype=np.float32)
    ffn_b1 = np.asarray(inputs["ffn_b1"], dtype=np.float32)
    ffn_w2 = np.asarray(inputs["ffn_w2"], dtype=np.float32)
    ffn_b2 = np.asarray(inputs["ffn_b2"], dtype=np.float32)
    ln_ff_g = np.asarray(inputs["ln_ff_g"], dtype=np.float32)
    ln_ff_b = np.asarray(inputs["ln_ff_b"], dtype=np.float32)
    ln_q_g = np.asarray(inputs["ln_q_g"], dtype=np.float32)
    ln_q_b = np.asarray(inputs["ln_q_b"], dtype=np.float32)

    # ---- Host-side exact folds of LN affine params into next matmuls ----
    mu_wf = mu_w * ln_q_g[None, :]
    mu_bf = mu_b + mu_w @ ln_q_b
    sig_wf = sigma_w * ln_q_g[None, :]
    sig_bf = sigma_b + sigma_w @ ln_q_b
    w1f = ffn_w1 * ln_ff_g[None, :]
    b1f = ffn_b1 + ffn_w1 @ ln_ff_b
    w1sum = w1f.sum(axis=1)
    musum = mu_wf.sum(axis=1)
    sigsum = sig_wf.sum(axis=1)

    # Device computes S*V' with S = Derivative_Erf(u) = 2/sqrt(pi)*exp(-u^2)
    Vs = (V * (np.sqrt(np.pi) / 2.0)).astype(np.float16)
    Kh = K.astype(np.float16)

    QT = np.ascontiguousarray(Q.T)                    # (N, B)
    qtf = QT.reshape(NCORES, 128, B).transpose(1, 0, 2)
    muwT = np.ascontiguousarray(mu_wf.T)              # (N, N)  [jn, j]
    sigwT = np.ascontiguousarray(sig_wf.T)
    w1T = np.ascontiguousarray(w1f.T)                 # (N, M)
    w2T = np.ascontiguousarray(ffn_w2.T)              # (M, N)

    nc = _build_module()

    in_maps = []
    for c in range(NCORES):
        jsl = slice(c * NSL, (c + 1) * NSL)
        msl = slice(c * MSL, (c + 1) * MSL)
        head = np.concatenate([
            (-mu_bf[jsl]).reshape(NSL, 1),
            sig_bf[jsl].reshape(NSL, 1),
            QT[jsl, :],
        ], axis=1)
        msw = np.stack([
            muwT[:, jsl].reshape(NCORES, 128, NSL).transpose(1, 0, 2),
            sigwT[:, jsl].reshape(NCORES, 128, NSL).transpose(1, 0, 2),
        ], axis=1)                                    # (128, 2, 8, NSL)
        ms2 = np.concatenate([musum[jsl], sigsum[jsl]]).reshape(1, 2 * NSL)
        in_maps.append({
            "Ks": np.ascontiguousarray(Kh[:, jsl, :].transpose(1, 0, 2)),
            "Vs": np.ascontiguousarray(Vs[:, jsl, :].transpose(1, 0, 2)),
            "HEAD": np.ascontiguousarray(head),
            "QTF": np.ascontiguousarray(qtf).astype(np.float16),
            "MSW": np.ascontiguousarray(msw).astype(np.float16),
            "MS2": np.ascontiguousarray(ms2),
            "W1T": np.ascontiguousarray(
                w1T[:, msl].reshape(NCORES, 128, MSL).transpose(1, 0, 2)
            ).astype(np.float16),
            "FFNB": np.ascontiguousarray(b1f[msl].reshape(MCH, 128).T),
            "W1S": np.ascontiguousarray(w1sum[msl]).reshape(1, MSL),
            "W2T": np.ascontiguousarray(
                w2T[msl, :].reshape(MCH, 128, N).transpose(1, 0, 2)
            ).astype(np.float16),
        })

    trace = os.environ.get("BASS_KERNEL_TRACE", "0") == "1"
    res = run_bass_kernel_spmd(
        nc, in_maps, core_ids=list(range(NCORES)), trace=trace
    )
    last_results = res

    x = np.concatenate([res.results[c]["XT"] for c in range(NCORES)],
                       axis=0).T + Q
    h = np.zeros((N, B), dtype=np.float32)
    for c in range(NCORES):
        h += res.results[c]["HP"]
    out = x + h.T + ffn_b2[None, :]
    return out.astype(np.float32)


# revision 9
# speedup vs baseline: 1.0211x; 1.0135x over previous
"""Bass/Trainium2 kernel for nn_KernelizedAttentionResBlock.

Sharding: n-token sharded attention (each of 8 cores owns 128 rows of n for
all batches), one AllGather of x^T (n-major, fp16 payload), m-sharded FFN,
host sums the 8 partial FFN outputs and adds the residual/bias.

Levers:
- K/V + weights fp16: the kernel is DMA-bound at ~360 GB/s/core, so halving
  HBM traffic halves phase-2 time.
- Gaussian kernel in ONE activation pass: Derivative_Erf(rs*K - mu*rs) =
  2/sqrt(pi) * exp(-0.5 (K-mu)^2/(sigma^2+1e-8)); sqrt(pi)/2 folded into V
  on the host. (tensor_tensor_reduce would fuse the reduce too but wedges
  the device, so S*V and the D-reduce are separate ops.)
- The 32 multiplies/reduces are spread over DVE, Pool (gpsimd multiplies)
  and the Act engine (Identity with accum_out, which stays inside the
  erf_derivative act table), keeping every engine under the DMA pace.
- Only {Square, Tanh, Derivative_Erf, Sigmoid, Identity, Copy} activations:
  3 act-table loads, all off the critical path (a dummy Sigmoid pinned
  behind phase 2 preloads the FFN table during the collective). rsqrt is
  computed on the DVE via the 0x5f3759df bit trick + 1 Newton step (the
  Ln/Exp route would thrash activation tables).
- Both LayerNorms are folded into the adjacent matmuls in transposed
  layout: ln(v)^T @ W = rstd_b * (v^T @ W - mean_b * rowsum(W)); the
  rank-1 mean correction is an extra matmul accumulated into the same
  PSUM and rstd_b is a PE-broadcast row multiplied in afterwards (via an
  SBUF copy: tensor ops may read only one PSUM operand).
- Head tensors first on the SP queue so phase 0/1 starts immediately; FFN
  weights issued after cc_in so their transfer hides inside the collective.
"""
import os
import sys

sys.path.insert(0, "/opt/trn_rl_repo")

import numpy as np

N = 1024
B = 32
D = 1024
M = 4096
NCORES = 8
NSL = N // NCORES
MSL = M // NCORES
MCH = MSL // 128
NB = 4
LN_EPS = 1e-5
MAGIC1 = 0x5F3759E0  # 0x5f3759df + 1 (for MAGIC - x == ~x + MAGIC+1)
SQH = float(np.sqrt(0.5))

_built = {}
last_results = None


def _build_module():
    if "nc" in _built:
        return _built["nc"]

    import concourse.bacc as bacc
    import concourse.mybir as mybir
    import concourse.tile as tile

    AF = mybir.ActivationFunctionType
    ALU = mybir.AluOpType
    f32 = mybir.dt.float32
    f16 = mybir.dt.float16
    i32 = mybir.dt.int32
    f8 = mybir.dt.float8e4

    nc = bacc.Bacc(trn_type="TRN2", num_devices=NCORES)

    Kd = nc.dram_tensor("Ks", (NSL, B, D), f16, kind="ExternalInput")
    Vd = nc.dram_tensor("Vs", (NSL, B, D), f16, kind="ExternalInput")
    # HEAD packs [-mu_b | sig_b | Q^T slice] along the free dim
    HEAD = nc.dram_tensor("HEAD", (128, 2 + B), f32, kind="ExternalInput")
    QTF = nc.dram_tensor("QTF", (128, NCORES, B), f16, kind="ExternalInput")
    MSW = nc.dram_tensor("MSW", (128, 2, NCORES, NSL), f16,
                         kind="ExternalInput")
    MS2 = nc.dram_tensor("MS2", (1, 2 * NSL), f32, kind="ExternalInput")
    W1T = nc.dram_tensor("W1T", (128, NCORES, MSL), f16, kind="ExternalInput")
    FFNB = nc.dram_tensor("FFNB", (128, MCH), f32, kind="ExternalInput")
    W1S = nc.dram_tensor("W1S", (1, MSL), f32, kind="ExternalInput")
    W2T = nc.dram_tensor("W2T", (128, MCH, N), f16, kind="ExternalInput")

    XTd = nc.dram_tensor("XT", (NSL, B), f32, kind="ExternalOutput")
    HPd = nc.dram_tensor("HP", (N, B), f32, kind="ExternalOutput")

    cc_in = nc.dram_tensor("cc_in", (NSL, B), f8, kind="Internal")
    cc_out = nc.dram_tensor(
        "cc_out", (N, B), f8, kind="Internal", addr_space="Shared"
    )

    def rsqrt_newton(pool, tag, v, steps, init, final_scale=1.0):
        """rsqrt(v) on the DVE: `init` is a same-shape f32 starting guess
        builder; 2 Newton steps; final_scale folded into the last step."""
        shape = list(v.shape)
        y = init
        for s in range(steps):
            k = final_scale if s == steps - 1 else 1.0
            t1 = pool.tile(shape, f32, tag=f"{tag}t1")
            nc.vector.tensor_mul(t1[:], y, y)
            t2 = pool.tile(shape, f32, tag=f"{tag}t2")
            nc.vector.tensor_mul(t2[:], t1[:], v[:])
            t3 = pool.tile(shape, f32, tag=f"{tag}t3")
            nc.vector.tensor_scalar(t3[:], t2[:], -0.5 * k, 1.5 * k,
                                    op0=ALU.mult, op1=ALU.add)
            yn = pool.tile(shape, f32, tag=f"{tag}n{s}")
            nc.vector.tensor_mul(yn[:], y, t3[:])
            y = yn[:]
        return y

    def rsqrt_bit(pool, tag, v, final_scale=1.0, steps=1):
        """Full-range rsqrt: quake init + Newton steps."""
        shape = list(v.shape)
        sh = pool.tile(shape, i32, tag=f"{tag}sh")
        nc.vector.tensor_scalar(sh[:], v[:].bitcast(i32), 1, None,
                                op0=ALU.logical_shift_right)
        y0i = pool.tile(shape, i32, tag=f"{tag}y0")
        nc.vector.tensor_scalar(y0i[:], sh[:], -1, MAGIC1 - 1,
                                op0=ALU.mult, op1=ALU.add)
        if steps == 0:
            return y0i[:].bitcast(f32)
        return rsqrt_newton(pool, tag, v, steps, y0i[:].bitcast(f32),
                            final_scale)

    with tile.TileContext(nc) as tc:
        with tc.tile_pool(name="const", bufs=1) as cst, \
             tc.tile_pool(name="small", bufs=1) as sm, \
             tc.tile_pool(name="kv", bufs=6) as kv, \
             tc.tile_pool(name="scr", bufs=4) as scr, \
             tc.tile_pool(name="psum", bufs=1, space="PSUM") as ps:

            # ---- head tensors first, then the K/V stream (SP queue) ---
            head = cst.tile([128, 2 + B], f32)
            nc.sync.dma_start(head[:], HEAD[:])
            qtf = cst.tile([128, NCORES, B], f16)
            nc.sync.dma_start(qtf[:], QTF[:])
            msw = cst.tile([128, 2, NCORES, NSL], f16)
            nc.sync.dma_start(msw[:], MSW[:])
            ms2 = cst.tile([1, 2 * NSL], f32)
            nc.sync.dma_start(ms2[:], MS2[:])

            BLOCKS = [(i * NB, NB) for i in range(B // NB - 2)]
            BLOCKS += [(24, 2), (26, 2), (28, 2), (30, 1), (31, 1)]
            kts, vts = [], []
            for b0, nb in BLOCKS:
                tn = max(nb, 2)
                kt = kv.tile([NSL, tn, D], f16, tag=f"kt{tn}")
                nc.sync.dma_start(kt[:, 0:nb, :], Kd[:, b0:b0 + nb, :])
                vt = kv.tile([NSL, tn, D], f16, tag=f"vt{tn}")
                nc.sync.dma_start(vt[:, 0:nb, :], Vd[:, b0:b0 + nb, :])
                kts.append(kt)
                vts.append(vt)

            # ---- head tensors on the Pool queue -----------------------
            nmb = head[:, 0:1]
            sb1 = head[:, 1:2]
            qts = head[:, 2:2 + B]

            ones16 = cst.tile([128, 1], f16)
            nc.vector.memset(ones16[:], 1.0)
            ones8 = cst.tile([128, 1], f8)
            nc.vector.memset(ones8[:], 1.0)
            ones32 = cst.tile([128, 1], f32)
            nc.vector.memset(ones32[:], 1.0)
            ones_row = cst.tile([1, 128], f32)
            nc.vector.memset(ones_row[:], 1.0)

            # ---------- Phase 0/1: LN(Q) folded into mu/sigma -----------
            qsq = sm.tile([128, NCORES, B], f32)
            nc.scalar.activation(qsq[:], qtf[:], AF.Square)
            qs_ps = ps.tile([1, B], f32, tag="pA")
            for c in range(NCORES):
                nc.tensor.matmul(qs_ps[:], ones16[:], qtf[:, c, :],
                                 start=(c == 0), stop=(c == NCORES - 1))
            qs2_ps = ps.tile([1, B], f32, tag="pB")
            for c in range(NCORES):
                nc.tensor.matmul(qs2_ps[:], ones32[:], qsq[:, c, :],
                                 start=(c == 0), stop=(c == NCORES - 1))
            negmean = sm.tile([1, B], f32)
            nc.vector.tensor_scalar_mul(negmean[:], qs_ps[:], -1.0 / N)
            msq = sm.tile([1, B], f32)
            nc.vector.tensor_mul(msq[:], negmean[:], negmean[:])
            varq = sm.tile([1, B], f32)
            nc.vector.tensor_scalar(varq[:], qs2_ps[:], 1.0 / N, LN_EPS,
                                    op0=ALU.mult, op1=ALU.add)
            varq2 = sm.tile([1, B], f32)
            nc.vector.tensor_sub(varq2[:], varq[:], msq[:])
            # q-row variance is ~1 (Q ~ N(0,1)): linear init + 2 Newton
            y0q = sm.tile([1, B], f32)
            nc.vector.tensor_scalar(y0q[:], varq2[:], -0.5, 1.5,
                                    op0=ALU.mult, op1=ALU.add)
            rstdq = rsqrt_newton(sm, "rq", varq2, 1, y0q[:])
            RSTD0 = ps.tile([128, B], f32, tag="pC")
            nc.tensor.matmul(RSTD0[:], ones_row[:], rstdq,
                             start=True, stop=True)
            rstd0_sb = sm.tile([128, B], f32)
            nc.vector.tensor_scalar_mul(rstd0_sb[:], RSTD0[:], 1.0)

            mu_ps = ps.tile([NSL, B], f32, tag="pA")
            nc.tensor.matmul(mu_ps[:], ms2[:, 0:NSL], negmean[:],
                             start=True, stop=False)
            for c in range(NCORES):
                nc.tensor.matmul(mu_ps[:], msw[:, 0, c, :], qtf[:, c, :],
                                 start=False, stop=(c == NCORES - 1))
            zmu = sm.tile([NSL, B], f32)
            nc.vector.tensor_mul(zmu[:], mu_ps[:], rstd0_sb[:])
            negmu = sm.tile([NSL, B], f32)
            nc.scalar.activation(negmu[:], zmu[:], AF.Tanh,
                                 bias=nmb, scale=-1.0)

            sig_ps = ps.tile([NSL, B], f32, tag="pB")
            nc.tensor.matmul(sig_ps[:], ms2[:, NSL:2 * NSL], negmean[:],
                             start=True, stop=False)
            for c in range(NCORES):
                nc.tensor.matmul(sig_ps[:], msw[:, 1, c, :], qtf[:, c, :],
                                 start=False, stop=(c == NCORES - 1))
            zsig = sm.tile([NSL, B], f32)
            nc.vector.tensor_mul(zsig[:], sig_ps[:], rstd0_sb[:])
            s2 = sm.tile([NSL, B], f32)
            nc.scalar.activation(s2[:], zsig[:], AF.Square, bias=sb1)
            s2e = sm.tile([NSL, B], f32)
            nc.vector.tensor_scalar_add(s2e[:], s2[:], 1e-8)
            # rs = sqrt(0.5/(sigma^2+1e-8)) — full-range bit-trick rsqrt
            rs = rsqrt_bit(sm, "rs", s2e, final_scale=SQH)
            nmr = sm.tile([NSL, B], f32)
            nc.vector.tensor_mul(nmr[:], negmu[:], rs)

            # ---------- Phase 2: x^T = sum_D S*V' + Q^T -----------------
            # tensor_tensor_reduce wedges the device; use mult + reduce,
            # spreading 12 of 32 reduces onto the idle Pool engine.
            AX = mybir.AxisListType
            xT = sm.tile([NSL, B], f32)
            for blk, (b0, nb) in enumerate(BLOCKS):
                kt, vt = kts[blk], vts[blk]
                for bi in range(nb):
                    b = b0 + bi
                    es = scr.tile([NSL, D], f16, tag="es")
                    nc.scalar.activation(es[:], kt[:, bi, :],
                                         AF.Derivative_Erf,
                                         bias=nmr[:, b:b + 1],
                                         scale=rs[:, b:b + 1])
                    sv = scr.tile([NSL, D], f16, tag="sv")
                    meng = nc.gpsimd if b % 4 == 1 else nc.vector
                    meng.tensor_mul(sv[:], es[:], vt[:, bi, :])
                    if b % 4 == 3:
                        # Act engine reduce: Identity with accumulator out
                        # (identity is in the erf_derivative table: no load)
                        ad = scr.tile([NSL, D], f16, tag="ad")
                        nc.scalar.activation(ad[:], sv[:], AF.Identity,
                                             accum_out=xT[:, b:b + 1])
                    else:
                        nc.vector.reduce_sum(xT[:, b:b + 1], sv[:],
                                             axis=AX.X)
            # x = A + Q fused with the f16 cast; XTd ships raw A and the
            # host adds the Q residual in f32.
            xh16 = sm.tile([NSL, B], f8)
            nc.vector.tensor_add(xh16[:], xT[:], qts)
            nc.sync.dma_start(cc_in[:], xh16[:])
            nc.sync.dma_start(XTd[:], xT[:])

            # preload the sigmoid act table during the collective window
            # (input xh16 pins it after phase 2 so the load can't be hoisted)
            sgd = sm.tile([NSL, 1], f32)
            nc.scalar.activation(sgd[:], xh16[:, 0:1], AF.Sigmoid)

            # FFN weights: transfers land inside the collective window.
            w1T = cst.tile([128, NCORES, MSL], f16)
            nc.sync.dma_start(w1T[:], W1T[:])
            ffnb = cst.tile([128, MCH], f32)
            nc.sync.dma_start(ffnb[:], FFNB[:])
            w1s = cst.tile([1, MSL], f32)
            nc.sync.dma_start(w1s[:], W1S[:])
            w2T = cst.tile([128, MCH, N], f16)
            nc.sync.dma_start(w2T[:], W2T[:])

            # ---------- Phase 3: AllGather x^T (n-major), LN, FFN -------
            nc.gpsimd.collective_compute(
                "AllGather", ALU.bypass,
                replica_groups=[list(range(NCORES))],
                ins=[cc_in[:]], outs=[cc_out[:]],
            )
            xg8 = sm.tile([128, NCORES, B], f8)
            nc.sync.dma_start(
                xg8[:], cc_out[:].rearrange("(c p) b -> p c b", p=128))
            xg16 = sm.tile([128, NCORES, B], f16)
            nc.vector.tensor_scalar_mul(xg16[:], xg8[:], 1.0)
            xsq = sm.tile([128, NCORES, B], f32)
            nc.vector.tensor_mul(xsq[:], xg8[:], xg8[:])
            s_ps = ps.tile([1, B], f32, tag="pA")
            for c in range(NCORES):
                nc.tensor.matmul(s_ps[:], ones8[:], xg8[:, c, :],
                                 start=(c == 0), stop=(c == NCORES - 1))
            s2_ps = ps.tile([1, B], f32, tag="pB")
            for c in range(NCORES):
                nc.tensor.matmul(s2_ps[:], ones32[:], xsq[:, c, :],
                                 start=(c == 0), stop=(c == NCORES - 1))
            negmx = sm.tile([1, B], f32)
            nc.vector.tensor_scalar_mul(negmx[:], s_ps[:], -1.0 / N)
            msqx = sm.tile([1, B], f32)
            nc.vector.tensor_mul(msqx[:], negmx[:], negmx[:])
            varx = sm.tile([1, B], f32)
            nc.vector.tensor_scalar(varx[:], s2_ps[:], 1.0 / N, LN_EPS,
                                    op0=ALU.mult, op1=ALU.add)
            varx2 = sm.tile([1, B], f32)
            nc.vector.tensor_sub(varx2[:], varx[:], msqx[:])
            rstdx = rsqrt_bit(sm, "rx", varx2, steps=0)
            RSTD1 = ps.tile([128, B], f32, tag="pC")
            nc.tensor.matmul(RSTD1[:], ones_row[:], rstdx,
                             start=True, stop=True)
            rstd1_sb = sm.tile([128, B], f32)
            nc.vector.tensor_scalar_mul(rstd1_sb[:], RSTD1[:], 1.0)

            # FFN: h1 = (x@w1 - mean*w1sum)*rstd + b1 ; silu = z*sigmoid(z)
            g1_sb = sm.tile([128, MCH, B], f16)
            for mi in range(MCH):
                h1_ps = ps.tile([128, B], f32, tag=f"p{chr(68 + mi)}")
                nc.tensor.matmul(h1_ps[:], w1s[:, mi * 128:(mi + 1) * 128],
                                 negmx[:], start=True, stop=False)
                for c in range(NCORES):
                    nc.tensor.matmul(h1_ps[:],
                                     w1T[:, c, mi * 128:(mi + 1) * 128],
                                     xg16[:, c, :],
                                     start=False, stop=(c == NCORES - 1))
                zpre = sm.tile([128, B], f32, tag=f"zp_{mi}")
                nc.vector.tensor_mul(zpre[:], h1_ps[:], rstd1_sb[:])
                sg = sm.tile([128, B], f32, tag=f"sg_{mi}")
                nc.scalar.activation(sg[:], zpre[:], AF.Sigmoid,
                                     bias=ffnb[:, mi:mi + 1])
                z = sm.tile([128, B], f32, tag=f"z_{mi}")
                nc.vector.tensor_scalar_add(z[:], zpre[:],
                                            ffnb[:, mi:mi + 1])
                nc.vector.tensor_mul(g1_sb[:, mi, :], z[:], sg[:])

            hp_sb = sm.tile([128, NCORES, B], f32)
            hpv = HPd[:].rearrange("(jn p) b -> p jn b", p=128)
            for jn in range(NCORES):
                hp_ps = ps.tile([128, B], f32, tag=f"p{chr(68 + jn % 4)}")
                for mi in range(MCH):
                    nc.tensor.matmul(hp_ps[:],
                                     w2T[:, mi, jn * 128:(jn + 1) * 128],
                                     g1_sb[:, mi, :],
                                     start=(mi == 0), stop=(mi == MCH - 1))
                if jn % 2 == 0:
                    nc.scalar.copy(hp_sb[:, jn, :], hp_ps[:])
                else:
                    nc.vector.tensor_scalar_mul(hp_sb[:, jn, :], hp_ps[:],
                                                1.0)
                if jn == 3:
                    nc.sync.dma_start(hpv[:, 0:4, :], hp_sb[:, 0:4, :])
            nc.sync.dma_start(hpv[:, 4:8, :], hp_sb[:, 4:8, :])

    nc.finalize()
    _built["nc"] = nc
    return nc


def kernel(**inputs):
    from concourse.bass_utils import run_bass_kernel_spmd

    global last_results

    Q = np.asarray(inputs["Q"], dtype=np.float32)
    K = np.asarray(inputs["K"], dtype=np.float32)
    V = np.asarray(inputs["V"], dtype=np.float32)
    mu_w = np.asarray(inputs["mu_w"], dtype=np.float32)
    mu_b = np.asarray(inputs["mu_b"], dtype=np.float32)
    sigma_w = np.asarray(inputs["sigma_w"], dtype=np.float32)
    sigma_b = np.asarray(inputs["sigma_b"], dtype=np.float32)
    ffn_w1 = np.asarray(inputs["ffn_w1"], dtype=np.float32)
    ffn_b1 = np.asarray(inputs["ffn_b1"], dtype=np.float32)
    ffn_w2 = np.asarray(inputs["ffn_w2"], dtype=np.float32)
    ffn_b2 = np.asarray(inputs["ffn_b2"], dtype=np.float32)
    ln_ff_g = np.asarray(inputs["ln_ff_g"], dtype=np.float32)
    ln_ff_b = np.asarray(inputs["ln_ff_b"], dtype=np.float32)
    ln_q_g = np.asarray(inputs["ln_q_g"], dtype=np.float32)
    ln_q_b = np.asarray(inputs["ln_q_b"], dtype=np.float32)

    # ---- Host-side exact folds of LN affine params into next matmuls ----
    mu_wf = mu_w * ln_q_g[None, :]
    mu_bf = mu_b + mu_w @ ln_q_b
    sig_wf = sigma_w * ln_q_g[None, :]
    sig_bf = sigma_b + sigma_w @ ln_q_b
    w1f = ffn_w1 * ln_ff_g[None, :]
    b1f = ffn_b1 + ffn_w1 @ ln_ff_b
    w1sum = w1f.sum(axis=1)
    musum = mu_wf.sum(axis=1)
    sigsum = sig_wf.sum(axis=1)

    # Device computes S*V' with S = Derivative_Erf(u) = 2/sqrt(pi)*exp(-u^2)
    Vs = (V * (np.sqrt(np.pi) / 2.0)).astype(np.float16)
    Kh = K.astype(np.float16)

    QT = np.ascontiguousarray(Q.T)                    # (N, B)
    qtf = QT.reshape(NCORES, 128, B).transpose(1, 0, 2)
    muwT = np.ascontiguousarray(mu_wf.T)              # (N, N)  [jn, j]
    sigwT = np.ascontiguousarray(sig_wf.T)
    w1T = np.ascontiguousarray(w1f.T)                 # (N, M)
    w2T = np.ascontiguousarray(ffn_w2.T)              # (M, N)

    nc = _build_module()

    in_maps = []
    for c in range(NCORES):
        jsl = slice(c * NSL, (c + 1) * NSL)
        msl = slice(c * MSL, (c + 1) * MSL)
        head = np.concatenate([
            (-mu_bf[jsl]).reshape(NSL, 1),
            sig_bf[jsl].reshape(NSL, 1),
            QT[jsl, :],
        ], axis=1)
        msw = np.stack([
            muwT[:, jsl].reshape(NCORES, 128, NSL).transpose(1, 0, 2),
            sigwT[:, jsl].reshape(NCORES, 128, NSL).transpose(1, 0, 2),
        ], axis=1)                                    # (128, 2, 8, NSL)
        ms2 = np.concatenate([musum[jsl], sigsum[jsl]]).reshape(1, 2 * NSL)
        in_maps.append({
            "Ks": np.ascontiguousarray(Kh[:, jsl, :].transpose(1, 0, 2)),
            "Vs": np.ascontiguousarray(Vs[:, jsl, :].transpose(1, 0, 2)),
            "HEAD": np.ascontiguousarray(head),
            "QTF": np.ascontiguousarray(qtf).astype(np.float16),
            "MSW": np.ascontiguousarray(msw).astype(np.float16),
            "MS2": np.ascontiguousarray(ms2),
            "W1T": np.ascontiguousarray(
                w1T[:, msl].reshape(NCORES, 128, MSL).transpose(1, 0, 2)
            ).astype(np.float16),
            "FFNB": np.ascontiguousarray(b1f[msl].reshape(MCH, 128).T),
            "W1S": np.ascontiguousarray(w1sum[msl]).reshape(1, MSL),
            "W2T": np.ascontiguousarray(
                w2T[msl, :].reshape(MCH, 128, N).transpose(1, 0, 2)
            ).astype(np.float16),
        })

    trace = os.environ.get("BASS_KERNEL_TRACE", "0") == "1"
    res = run_bass_kernel_spmd(
        nc, in_maps, core_ids=list(range(NCORES)), trace=trace
    )
    last_results = res

    x = np.concatenate([res.results[c]["XT"] for c in range(NCORES)],
                       axis=0).T + Q
    h = np.zeros((N, B), dtype=np.float32)
    for c in range(NCORES):
        h += res.results[c]["HP"]
    out = x + h.T + ffn_b2[None, :]
    return out.astype(np.float32)
